# revision 1
# baseline (speedup 1.0000x reference)
"""3x3 conv via 1D Winograd F(4,3) along W as 18-matmul bands on TRN2.

Full inputs: x [32, 128, 56, 56] f32, w [1152, 256] f32 (row = c*9 + kh*3 + kw).
Full output: [32, 256, 56, 56] f32. Data-parallel: 4 images per core, 8 cores.

The W-direction 3-tap conv is Winograd-transformed with m=4: each output
4-column tile wt consumes the 6-point window x[4wt-1 .. 4wt+4] through the
B^T data transform (host-computed, 6 planes), contracted against
host-pre-transformed weights U = G g (6 planes x 3 vertical taps), with the
vertical taps accumulated in PSUM. The PE streams 3*6*14 = 252 columns per
4*56 output pixels instead of 9*56 = 504: a 2x reduction in TensorE time.

The device returns the six m-planes in fp16; the host applies the exact
4x6 A^T inverse (0.3% of the FLOPs) and interleaves. On device each band
is just 18 matmuls + two plane-triple PSUM->SBUF fp16 copies (ScalarE
planes 0-2, VectorE planes 3-5) which are the only PSUM readers, so PSUM
buffer reuse never waits on long chains.
"""

import numpy as np

import concourse.bass as bass  # noqa: F401  (registers AP types)
import concourse.mybir as mybir
import concourse.tile as tile
from concourse import bacc, bass_utils

B, C, H, W = 32, 128, 56, 56
COUT = 256
NCORES = 8
BPC = B // NCORES  # images per core
NP = 6  # winograd points per tile
WT = W // 4  # 14 column tiles
HP = H + 2  # D rows: output row i needs D rows i..i+2 (x rows i-1..i+1)
R = 14  # output rows per band
NB = H // R  # bands per (image, oc-half)
NF = R * WT  # matmul free size per plane (196)
PST = 256  # psum plane stride (f32); 6 planes = 3 banks
F32 = mybir.dt.float32
F16 = mybir.dt.float16
BF16 = mybir.dt.bfloat16
MOV = mybir.dt.float16
MOV_NP = np.float16

GM = np.array(
    [[1 / 4, 0, 0], [-1 / 6, -1 / 6, -1 / 6], [-1 / 6, 1 / 6, -1 / 6],
     [1 / 24, 1 / 12, 1 / 6], [1 / 24, -1 / 12, 1 / 6], [0, 0, 1]],
    dtype=np.float32)
BT = np.array(
    [[4, 0, -5, 0, 1, 0], [0, -4, -4, 1, 1, 0], [0, 4, -4, -1, 1, 0],
     [0, -2, -1, 2, 1, 0], [0, 2, -1, -2, 1, 0], [0, 4, 0, -5, 0, 1]],
    dtype=np.float32)
AT = np.array(
    [[1, 1, 1, 1, 1, 0], [0, 1, -1, 2, -2, 0], [0, 1, 1, 4, 4, 0],
     [0, 1, -1, 8, -8, 1]], dtype=np.float32)

_cached_nc = None


def _build():
    nc = bacc.Bacc(None, target_bir_lowering=False)
    d = nc.dram_tensor("d", [BPC, C, NP, HP, WT], MOV, kind="ExternalInput")
    # host pre-transformed weights: [oc_half, c, p, kh, 128]
    w = nc.dram_tensor("w", [2, C, NP, 3, 128], MOV, kind="ExternalInput")
    # band-major so each band's DMA is one contiguous chunk per partition
    out = nc.dram_tensor("out", [BPC, COUT, NB, NP, R, WT], F16,
                         kind="ExternalOutput")

    with tile.TileContext(nc) as tc:
        with (
            tc.tile_pool(name="wpool", bufs=1) as wpool,
            tc.tile_pool(name="dpool", bufs=4) as dpool,
            tc.tile_pool(name="opool", bufs=3) as opool,
            tc.tile_pool(name="pspool", bufs=2, space="PSUM") as pspool,
        ):
            # PE warmup: cover the preamble-to-first-data window so the HAM
            # clock gate is warming while the input DMA runs.
            NWARM = 13
            warm = wpool.tile([C, 448], BF16)
            nc.vector.memset(warm[:], 0.0)
            wpsum = pspool.tile([16, 448], F32, tag="pa", name="warm_psum")
            for i in range(NWARM):
                nc.tensor.matmul(wpsum[:], warm[:, :16], warm[:],
                                 start=(i == 0), stop=(i == NWARM - 1))

            # load order gates the first real matmul: och0 weights, then
            # per-plane first-two-band D rows, then the rest
            # weights on the sync ring in parallel with D on the scalar
            # ring; all image loads issued up front (4 D buffers)
            wbuf = wpool.tile([C, 2, NP, 3, 128], MOV)
            D0 = dpool.tile([C, NP, HP, WT], MOV, tag="D", name="D0")
            HS = 2 * R + 2
            nc.sync.dma_start(wbuf[:, 0], w[0])
            for p in range(NP):
                nc.scalar.dma_start(D0[:, p, 0:HS, :], d[0, :, p, 0:HS, :])
            nc.sync.dma_start(wbuf[:, 1], w[1])
            nc.scalar.dma_start(D0[:, :, HS:HP, :], d[0, :, :, HS:HP, :])
            Dt = [D0]
            for b in range(1, BPC):
                Db = dpool.tile([C, NP, HP, WT], MOV, tag="D", name=f"D{b}")
                nc.scalar.dma_start(Db[:], d[b])
                Dt.append(Db)

            for b in range(BPC):
                D = Dt[b]

                for och in range(2):
                    OB = opool.tile([C, NB, NP, R, WT], F16, tag="ob",
                                    name=f"ob{b}_{och}")
                    for t in range(NB):
                        r0 = t * R
                        band = (b * 2 + och) * NB + t
                        # 6 planes at 256-f32 stride across two 2-bank
                        # tiles (one reader each -> early WAR release)
                        PA = pspool.tile([C, 3, PST], F32, tag="pa",
                                         name=f"pa{band}")
                        PB = pspool.tile([C, 3, PST], F32, tag="pb",
                                         name=f"pb{band}")
                        for p in range(NP):
                            dst = (PA if p < 3 else PB)[:, p % 3, 0:NF]
                            for kh in range(3):
                                nc.tensor.matmul(
                                    dst,
                                    wbuf[:, och, p, kh, :],
                                    D[:, p, r0 + kh : r0 + kh + R, :],
                                    start=(kh == 0),
                                    stop=(kh == 2),
                                )
                        # evacuate the m-planes to fp16; these copies are
                        # the only PSUM readers. The very last band DMAs
                        # per plane-triple so only 150KB trails the end.
                        last = band == 2 * BPC * NB - 1
                        ocr = slice(och * 128, (och + 1) * 128)
                        if last:
                            nc.scalar.copy(out=OB[:, t, 0:3],
                                           in_=PA[:, :, 0:NF])
                            nc.sync.dma_start(out[b, ocr, t, 0:3],
                                              OB[:, t, 0:3])
                            nc.vector.tensor_copy(out=OB[:, t, 3:6],
                                                  in_=PB[:, :, 0:NF])
                            nc.sync.dma_start(out[b, ocr, t, 3:6],
                                              OB[:, t, 3:6])
                        else:
                            nc.scalar.copy(out=OB[:, t, 0:3],
                                           in_=PA[:, :, 0:NF])
                            nc.vector.tensor_copy(out=OB[:, t, 3:6],
                                                  in_=PB[:, :, 0:NF])
                            last_och = b == BPC - 1 and och == 1
                            if last_och and t == 2:
                                nc.sync.dma_start(out[b, ocr, t : t + 1],
                                                  OB[:, t : t + 1])
                            elif t % 2:
                                nc.sync.dma_start(out[b, ocr, t - 1 : t + 1],
                                                  OB[:, t - 1 : t + 1])
    nc.compile()
    return nc


def _get_nc():
    global _cached_nc
    if _cached_nc is None:
        _cached_nc = _build()
    return _cached_nc


def _host_weights(w):
    """w [1152, 256] f32 -> [oc_half, c, p, kh, 128] fp16 G-transformed."""
    g = np.asarray(w, dtype=np.float32).reshape(C, 3, 3, COUT)
    U = np.einsum("pk,chko->pcho", GM, g)  # [p, c, kh, oc]
    return np.ascontiguousarray(
        U.reshape(NP, C, 3, 2, 128).transpose(3, 1, 0, 2, 4)
    ).astype(MOV_NP)


def _host_fwd(x):
    """x [B, C, H, W] f32 -> D [B, C, 6, 58, 14] fp16 (F(4,3) B^T transform)."""
    x = np.asarray(x, dtype=np.float32)
    xw = np.pad(x, ((0, 0), (0, 0), (0, 0), (1, 3)))
    win = np.stack([xw[..., 4 * t : 4 * t + 6] for t in range(WT)], axis=-2)
    # win: [B, C, H, WT, 6]
    D = np.zeros((B, C, NP, HP, WT), MOV_NP)
    D[:, :, :, 1 : H + 1, :] = np.einsum("pj,bchwj->bcphw", BT, win)
    return D


def run(x, w, trace=False, **spmd_kwargs):
    nc = _get_nc()
    dfull = _host_fwd(x)
    w2 = _host_weights(w)
    in_maps = [
        {"d": dfull[i * BPC : (i + 1) * BPC], "w": w2} for i in range(NCORES)
    ]
    res = bass_utils.run_bass_kernel_spmd(
        nc, in_maps, core_ids=list(range(NCORES)), trace=trace, **spmd_kwargs
    )
    # dev out m-planes [BPC, 256, NB, 6, R, 14] -> A^T inverse -> full
    m = np.concatenate([r["out"] for r in res.results], axis=0).astype(np.float32)
    y = np.einsum("jp,botprw->botrwj", AT, m)  # [B, 256, NB, R, WT, 4]
    full = np.ascontiguousarray(y.reshape(B, COUT, H, W))
    return full, res


def kernel(x, w):
    return run(x, w)[0]



# revision 6
# speedup vs baseline: 1.0541x; 1.0541x over previous
"""3x3 conv via 1D Winograd F(8,3) along W as full-height matmul bands on TRN2.

Full inputs: x [32, 128, 56, 56] f32, w [1152, 256] f32 (row = c*9 + kh*3 + kw).
Full output: [32, 256, 56, 56] f32. Data-parallel: 4 images per core, 8 cores.

The W-direction 3-tap conv is Winograd-transformed with m=8: each output
8-column tile consumes a 10-point window of x through the B^T data transform
(host-computed, 10 planes, nodes {0,±1/2,±3/4,±4/3,±2,inf}), contracted
against host-pre-transformed weights U = G g (10 planes x 3 vertical taps),
with the vertical taps accumulated in PSUM. Per (image, oc-half) each plane
is ONE matmul band of N = 56*7 = 392 columns (whole image height), so the
PE streams 3*10*392 cycles per 128*3136 outputs: 2.4x fewer columns than
direct convolution and near-zero per-matmul issue overhead.

PSUM: one plane per 2KB bank, 10 planes cycling 8 banks (bufs=1 tags); the
only PSUM readers are per-plane fp16 evacuation copies alternating between
ScalarE and VectorE, so bank reuse never stalls the matmul stream. The
device returns the ten m-planes in fp16; the host applies the exact 8x10
A^T inverse (small fraction of FLOPs) and interleaves.

Startup: weights stream on the sync HWDGE ring, data planes on the scalar
ring, fine-grained chunks first so plane p of image 0 lands just ahead of
its matmuls while warmup matmuls cover the preamble and HAM clock-gate ramp.
"""

import numpy as np

import concourse.bass as bass  # noqa: F401  (registers AP types)
import concourse.mybir as mybir
import concourse.tile as tile
from concourse import bacc, bass_utils

B, C, H, W = 32, 128, 56, 56
COUT = 256
NCORES = 8
BPC = B // NCORES  # images per core
M = 8  # winograd output tile size
NP = M + 2  # 10 winograd points/planes
WT = W // M  # 7 column tiles
HP = H + 2  # D rows: output row h needs D rows h..h+2 (x rows h-1..h+1)
R = H  # single full-height band
N = R * WT  # matmul free size per plane (392)
NODES = [0.0, 0.5, -0.5, 0.75, -0.75, 4.0 / 3.0, -4.0 / 3.0, 2.0, -2.0]
F32 = mybir.dt.float32
F16 = mybir.dt.float16
BF16 = mybir.dt.bfloat16
MOV = mybir.dt.float16
MOV_NP = np.float16


def _transforms():
    """Toom-Cook correlation transform (transposition principle).

    y = AT ((G g) . (BT d)) computes y_i = sum_k g_k d_{i+k} exactly:
    G/AT are node-power evaluations (+ leading-coeff rows for the inf
    node), BT = inv(E)^T with E the coefficient-evaluation matrix.
    """
    n, r, m = NP, 3, M
    a = np.array(NODES, np.float64)
    G = np.zeros((n, r))
    G[: n - 1] = a[:, None] ** np.arange(r)[None, :]
    G[n - 1, r - 1] = 1.0
    AT = np.zeros((m, n))
    AT[:, : n - 1] = a[None, :] ** np.arange(m)[:, None]
    AT[m - 1, n - 1] = 1.0
    E = np.zeros((n, n))
    E[: n - 1] = a[:, None] ** np.arange(n)[None, :]
    E[n - 1, n - 1] = 1.0
    BT = np.linalg.inv(E).T
    return (BT.astype(np.float32), G.astype(np.float32), AT.astype(np.float32))


BT32, G32, AT32 = _transforms()

_cached_nc = None


def _build():
    nc = bacc.Bacc(None, target_bir_lowering=False)
    # c-major so every DMA slice matches the SBUF tile's axis order
    d = nc.dram_tensor("d", [C, BPC, NP, HP, WT], MOV, kind="ExternalInput")
    # host pre-transformed weights: [oc_half, c, p, kh, 128]
    w = nc.dram_tensor("w", [2, C, NP, 3, 128], MOV, kind="ExternalInput")
    out = nc.dram_tensor("out", [BPC, COUT, NP, N], F16, kind="ExternalOutput")

    with tile.TileContext(nc) as tc:
        with (
            tc.tile_pool(name="wpool", bufs=1) as wpool,
            tc.tile_pool(name="dpool", bufs=1) as dpool,
            tc.tile_pool(name="opool", bufs=3) as opool,
            tc.tile_pool(name="pspool", bufs=1, space="PSUM") as pspool,
        ):
            # input DMAs: weights on the sync ring, data on the scalar ring,
            # fine-grained at the front so image-0 planes land in matmul order
            wbuf = wpool.tile([C, 2, NP, 3, 128], MOV)
            DD = dpool.tile([C, BPC, NP, HP, WT], MOV, tag="D")
            nc.sync.dma_start(wbuf[:, 0, 0], w[0, :, 0])
            nc.scalar.dma_start(DD[:, 0, 0], d[:, 0, 0])
            nc.sync.dma_start(wbuf[:, 0, 1], w[0, :, 1])
            nc.scalar.dma_start(DD[:, 0, 1], d[:, 0, 1])
            nc.sync.dma_start(wbuf[:, 0, 2:5], w[0, :, 2:5])
            nc.scalar.dma_start(DD[:, 0, 2:5], d[:, 0, 2:5])
            nc.sync.dma_start(wbuf[:, 0, 5:10], w[0, :, 5:10])
            nc.scalar.dma_start(DD[:, 0, 5:10], d[:, 0, 5:10])
            nc.sync.dma_start(wbuf[:, 1, 0:5], w[1, :, 0:5])
            nc.scalar.dma_start(DD[:, 1:4], d[:, 1:4])
            nc.sync.dma_start(wbuf[:, 1, 5:10], w[1, :, 5:10])

            # PE warmup: cover the preamble-to-first-data window and start
            # the HAM clock-gate busy window early
            NWARM = 6
            warm = wpool.tile([C, 448], BF16)
            nc.vector.memset(warm[:], 0.0)
            wps = pspool.tile([C, 512], F32, tag="b7", name="warm")
            for i in range(NWARM):
                nc.tensor.matmul(wps[:16, 0:448], warm[:, :16], warm[:],
                                 start=(i == 0), stop=(i == NWARM - 1))

            s = 0
            for b in range(BPC):
                for och in range(2):
                    OB = opool.tile([C, NP, N], F16, tag="ob",
                                    name=f"ob{b}{och}")
                    ocr = slice(och * 128, (och + 1) * 128)
                    last = b == BPC - 1 and och == 1
                    for p in range(NP):
                        ps = pspool.tile([C, 512], F32, tag=f"b{s % 8}",
                                         name=f"ps{s}")
                        for kh in range(3):
                            nc.tensor.matmul(
                                ps[:, 0:N],
                                wbuf[:, och, p, kh, :],
                                DD[:, b, p, kh : kh + R, :],
                                start=(kh == 0),
                                stop=(kh == 2),
                            )
                        if s % 2 == 0:
                            nc.scalar.copy(out=OB[:, p], in_=ps[:, 0:N])
                        else:
                            nc.vector.tensor_copy(out=OB[:, p], in_=ps[:, 0:N])
                        if last and p % 2 == 1:
                            # drain the last oc-half in plane pairs across
                            # both HWDGE rings so only ~200KB trails the end
                            ring = nc.sync if (p // 2) % 2 == 0 else nc.scalar
                            ring.dma_start(out[b, ocr, p - 1 : p + 1],
                                           OB[:, p - 1 : p + 1])
                        s += 1
                    if not last:
                        nc.sync.dma_start(out[b, ocr], OB[:])
    nc.compile()
    return nc


def _get_nc():
    global _cached_nc
    if _cached_nc is None:
        _cached_nc = _build()
    return _cached_nc


def _host_weights(w):
    """w [1152, 256] f32 -> [oc_half, c, p, kh, 128] fp16 G-transformed."""
    g = np.asarray(w, dtype=np.float32).reshape(C, 3, 3, COUT)
    U = np.einsum("pk,chko->pcho", G32, g)  # [NP, c, kh, oc]
    return np.ascontiguousarray(
        U.reshape(NP, C, 3, 2, 128).transpose(3, 1, 0, 2, 4)
    ).astype(MOV_NP)


def _host_fwd(x):
    """x [B, C, H, W] f32 -> D [C, B, NP, 58, WT] fp16 (B^T transform)."""
    x = np.asarray(x, dtype=np.float32)
    xw = np.pad(x, ((0, 0), (0, 0), (0, 0), (1, 1)))
    win = np.stack([xw[..., M * t : M * t + NP] for t in range(WT)], axis=-2)
    # win: [B, C, H, WT, NP]
    D = np.zeros((C, B, NP, HP, WT), MOV_NP)
    D[:, :, :, 1 : H + 1, :] = np.einsum("pj,bchtj->cbpht", BT32, win)
    return D


def run(x, w, trace=False, **spmd_kwargs):
    nc = _get_nc()
    dfull = _host_fwd(x)
    w2 = _host_weights(w)
    in_maps = [
        {"d": np.ascontiguousarray(dfull[:, i * BPC : (i + 1) * BPC]), "w": w2}
        for i in range(NCORES)
    ]
    res = bass_utils.run_bass_kernel_spmd(
        nc, in_maps, core_ids=list(range(NCORES)), trace=trace, **spmd_kwargs
    )
    # dev out m-planes [BPC, 256, NP, 392] -> A^T inverse -> full output
    m = np.concatenate([r["out"] for r in res.results], axis=0).astype(np.float32)
    m = m.reshape(B, COUT, NP, R, WT)
    y = np.einsum("jp,bopht->bohtj", AT32, m)  # [B, 256, H, WT, M]
    full = np.ascontiguousarray(y.reshape(B, COUT, H, W))
    return full, res


def kernel(x, w):
    return run(x, w)[0]


# revision 8
# speedup vs baseline: 1.1771x; 1.1166x over previous
"""3x3 conv via 1D Winograd F(8,3) along W as full-height matmul bands on TRN2.

Full inputs: x [32, 128, 56, 56] f32, w [1152, 256] f32 (row = c*9 + kh*3 + kw).
Full output: [32, 256, 56, 56] f32. Data-parallel: 4 images per core, 8 cores.

The W-direction 3-tap conv is Winograd-transformed with m=8: each output
8-column tile consumes a 10-point window of x through the B^T data transform
(host-computed, 10 planes, nodes {0,±1/2,±3/4,±4/3,±2,inf}), contracted
against host-pre-transformed weights U = G g (10 planes x 3 vertical taps),
with the vertical taps accumulated in PSUM. Per (image, oc-half) each plane
is ONE matmul band of N = 56*7 = 392 columns (whole image height), so the
PE streams 3*10*392 cycles per 128*3136 outputs: 2.4x fewer columns than
direct convolution and near-zero per-matmul issue overhead.

PSUM: one plane per 2KB bank, 10 planes cycling 8 banks (bufs=1 tags); the
only PSUM readers are per-plane fp16 evacuation copies alternating between
ScalarE and VectorE, so bank reuse never stalls the matmul stream. The
device returns the ten m-planes in fp16; the host applies the exact 8x10
A^T inverse (small fraction of FLOPs) and interleaves.

Startup: weights stream on the sync HWDGE ring, data planes on the scalar
ring, fine-grained chunks first so plane p of image 0 lands just ahead of
its matmuls while warmup matmuls cover the preamble and HAM clock-gate ramp.
"""

import numpy as np

import concourse.bass as bass  # noqa: F401  (registers AP types)
import concourse.mybir as mybir
import concourse.tile as tile
from concourse import bacc, bass_utils

B, C, H, W = 32, 128, 56, 56
COUT = 256
NCORES = 8
BPC = B // NCORES  # images per core
M = 8  # winograd output tile size
NP = M + 2  # 10 winograd points/planes
WT = W // M  # 7 column tiles
HP = H + 2  # D rows: output row h needs D rows h..h+2 (x rows h-1..h+1)
R = H  # single full-height band
N = R * WT  # matmul free size per plane (392)
NODES = [0.0, 0.5, -0.5, 0.75, -0.75, 4.0 / 3.0, -4.0 / 3.0, 2.0, -2.0]
F32 = mybir.dt.float32
F16 = mybir.dt.float16
BF16 = mybir.dt.bfloat16
MOV = mybir.dt.float16
MOV_NP = np.float16


def _transforms():
    """Toom-Cook correlation transform (transposition principle).

    y = AT ((G g) . (BT d)) computes y_i = sum_k g_k d_{i+k} exactly:
    G/AT are node-power evaluations (+ leading-coeff rows for the inf
    node), BT = inv(E)^T with E the coefficient-evaluation matrix.
    """
    n, r, m = NP, 3, M
    a = np.array(NODES, np.float64)
    G = np.zeros((n, r))
    G[: n - 1] = a[:, None] ** np.arange(r)[None, :]
    G[n - 1, r - 1] = 1.0
    AT = np.zeros((m, n))
    AT[:, : n - 1] = a[None, :] ** np.arange(m)[:, None]
    AT[m - 1, n - 1] = 1.0
    E = np.zeros((n, n))
    E[: n - 1] = a[:, None] ** np.arange(n)[None, :]
    E[n - 1, n - 1] = 1.0
    BT = np.linalg.inv(E).T
    return (BT.astype(np.float32), G.astype(np.float32), AT.astype(np.float32))


BT32, G32, AT32 = _transforms()

_cached_nc = None


def _build():
    nc = bacc.Bacc(None, target_bir_lowering=False)
    # c-major so every DMA slice matches the SBUF tile's axis order
    d = nc.dram_tensor("d", [C, BPC, NP, HP, WT], MOV, kind="ExternalInput")
    # host pre-transformed weights: [oc_half, c, p, kh, 128]
    w = nc.dram_tensor("w", [2, C, NP, 3, 128], MOV, kind="ExternalInput")
    out = nc.dram_tensor("out", [BPC, COUT, NP, N], F16, kind="ExternalOutput")

    with tile.TileContext(nc) as tc:
        with (
            tc.tile_pool(name="wpool", bufs=1) as wpool,
            tc.tile_pool(name="dpool", bufs=1) as dpool,
            tc.tile_pool(name="opool", bufs=3) as opool,
            tc.tile_pool(name="pspool", bufs=1, space="PSUM") as pspool,
        ):
            # input DMAs: och0 weights alone on the sync ring (small, fast);
            # everything else on the scalar ring in exact consumption order
            # so no late chunk can starve an earlier-needed one (the SDMA
            # engines round-robin queues at packet granularity, so a big
            # transfer on one queue throttles the other queue's chunks)
            wbuf = wpool.tile([C, 2, NP, 3, 128], MOV)
            DD = dpool.tile([C, BPC, NP, HP, WT], MOV, tag="D")
            nc.sync.dma_start(wbuf[:, 0, 0], w[0, :, 0])
            nc.scalar.dma_start(DD[:, 0, 0], d[:, 0, 0])
            nc.sync.dma_start(wbuf[:, 0, 1], w[0, :, 1])
            nc.scalar.dma_start(DD[:, 0, 1], d[:, 0, 1])
            nc.sync.dma_start(wbuf[:, 0, 2:5], w[0, :, 2:5])
            nc.scalar.dma_start(DD[:, 0, 2:5], d[:, 0, 2:5])
            nc.sync.dma_start(wbuf[:, 0, 5:10], w[0, :, 5:10])
            nc.scalar.dma_start(DD[:, 0, 5:10], d[:, 0, 5:10])
            nc.scalar.dma_start(wbuf[:, 1, 0:5], w[1, :, 0:5])
            nc.scalar.dma_start(wbuf[:, 1, 5:10], w[1, :, 5:10])
            # D1-3 triggers are emitted inside the (b0, och0) loop below so
            # the scalar engine issues the first PSUM copies without delay

            # PE warmup: cover the preamble-to-first-data window and start
            # the HAM clock-gate busy window early
            NWARM = 6
            warm = wpool.tile([C, 448], BF16)
            nc.vector.memset(warm[:], 0.0)
            wps = pspool.tile([C, 2, 512], F32, tag="b3", name="warm")
            for i in range(NWARM):
                nc.tensor.matmul(wps[:16, 0, 0:448], warm[:, :16], warm[:],
                                 start=(i == 0), stop=(i == NWARM - 1))

            spp = 0  # global plane-pair index
            for b in range(BPC):
                for och in range(2):
                    OB = opool.tile([C, NP, N], F16, tag="ob",
                                    name=f"ob{b}{och}")
                    ocr = slice(och * 128, (och + 1) * 128)
                    last = b == BPC - 1 and och == 1
                    for pp in range(NP // 2):
                        # two planes per PSUM tile (adjacent banks), one
                        # evacuation copy per pair
                        ps = pspool.tile([C, 2, 512], F32, tag=f"b{spp % 4}",
                                         name=f"ps{spp}")
                        for sub in range(2):
                            p = 2 * pp + sub
                            for kh in range(3):
                                nc.tensor.matmul(
                                    ps[:, sub, 0:N],
                                    wbuf[:, och, p, kh, :],
                                    DD[:, b, p, kh : kh + R, :],
                                    start=(kh == 0),
                                    stop=(kh == 2),
                                )
                        dst = OB[:, 2 * pp : 2 * pp + 2]
                        if spp % 2 == 0:
                            nc.scalar.copy(out=dst, in_=ps[:, :, 0:N])
                        else:
                            nc.vector.tensor_copy(out=dst, in_=ps[:, :, 0:N])
                        if b == 0 and och == 0 and pp >= 2:
                            # rest-of-input triggers sit here so the scalar
                            # engine issues the first copies without delay
                            bb = pp - 1
                            nc.scalar.dma_start(DD[:, bb], d[:, bb])
                        if last:
                            # drain the last oc-half in plane pairs across
                            # both HWDGE rings so only ~200KB trails the end
                            ring = nc.sync if spp % 2 == 0 else nc.scalar
                            ring.dma_start(out[b, ocr, 2 * pp : 2 * pp + 2],
                                           OB[:, 2 * pp : 2 * pp + 2])
                        spp += 1
                    if not last:
                        ring = nc.sync if och == 0 else nc.scalar
                        ring.dma_start(out[b, ocr], OB[:])
    nc.compile()
    return nc


def _get_nc():
    global _cached_nc
    if _cached_nc is None:
        _cached_nc = _build()
    return _cached_nc


def _host_weights(w):
    """w [1152, 256] f32 -> [oc_half, c, p, kh, 128] fp16 G-transformed."""
    g = np.asarray(w, dtype=np.float32).reshape(C, 3, 3, COUT)
    U = np.einsum("pk,chko->pcho", G32, g)  # [NP, c, kh, oc]
    return np.ascontiguousarray(
        U.reshape(NP, C, 3, 2, 128).transpose(3, 1, 0, 2, 4)
    ).astype(MOV_NP)


def _host_fwd(x):
    """x [B, C, H, W] f32 -> D [C, B, NP, 58, WT] fp16 (B^T transform)."""
    x = np.asarray(x, dtype=np.float32)
    xw = np.pad(x, ((0, 0), (0, 0), (0, 0), (1, 1)))
    win = np.stack([xw[..., M * t : M * t + NP] for t in range(WT)], axis=-2)
    # win: [B, C, H, WT, NP]
    D = np.zeros((C, B, NP, HP, WT), MOV_NP)
    D[:, :, :, 1 : H + 1, :] = np.einsum("pj,bchtj->cbpht", BT32, win)
    return D


def run(x, w, trace=False, **spmd_kwargs):
    nc = _get_nc()
    dfull = _host_fwd(x)
    w2 = _host_weights(w)
    in_maps = [
        {"d": np.ascontiguousarray(dfull[:, i * BPC : (i + 1) * BPC]), "w": w2}
        for i in range(NCORES)
    ]
    res = bass_utils.run_bass_kernel_spmd(
        nc, in_maps, core_ids=list(range(NCORES)), trace=trace, **spmd_kwargs
    )
    # dev out m-planes [BPC, 256, NP, 392] -> A^T inverse -> full output
    m = np.concatenate([r["out"] for r in res.results], axis=0).astype(np.float32)
    m = m.reshape(B, COUT, NP, R, WT)
    y = np.einsum("jp,bopht->bohtj", AT32, m)  # [B, 256, H, WT, M]
    full = np.ascontiguousarray(y.reshape(B, COUT, H, W))
    return full, res


def kernel(x, w):
    return run(x, w)[0]
